# revision 14
# baseline (speedup 1.0000x reference)
"""Binarize kernel for Trainium2 (8 NeuronCores, SPMD row-sharded).

Reference semantics (per row/channel i of x[4096, 16384]):
    alpha_i = sum(|x_i|) / count(x_i != 0)
    out[i,j] = (+1 if x[i,j] > 0 else -1) * alpha_i

Sharding: rows split evenly across 8 cores (512 rows each), no
communication needed.  Built on bacc.Bacc (NOT plain bass.Bass): Bacc's
compile pipeline legalizes TRN2's one-sync-wait-per-instruction limit
by splitting excess waits onto EventSemaphore instructions.

Per-core plan (rows-on-partitions; 4 row-blocks of 128 rows; each
row-block processed in 4 col-chunks of 4096):
  - DMA in each chunk xc.
  - ACT: Abs(xc) -> scratch(bf16), accum_out -> abssum partials.
  - ACT: Sign(xc) -> scratch(bf16), accum_out -> sgnsum partials
    (= npos - nneg; zeros contribute 0).
  - DVE: mc(bf16) = (xc is_gt 0) in {0,1}, accum_out -> npos partials.
  - count = 2*npos - sgnsum = npos + nneg = #nonzero (exact in fp32),
    alpha2 = 2*abssum/count, na = -alpha (x2 / x-0.5 are exact).
  - DVE: oc = mc * alpha2 + na  -> {+alpha, -alpha} exactly.
  - DMA out oc.
x is read from HBM exactly once and out written once (64 MiB/core
total -> memory-roofline bound at ~360 GB/s/core).
"""

import numpy as np
from contextlib import ExitStack

import concourse.bacc as bacc
import concourse.bass as bass
import concourse.mybir as mybir
import concourse.tile as tile
from concourse.bass_utils import run_bass_kernel_spmd

N_CORES = 8
ROWS, COLS = 4096, 16384
R = ROWS // N_CORES  # 512 rows per core
P = 128              # SBUF partitions
RB = R // P          # 4 row-blocks per core
CHUNK = 4096
NCH = COLS // CHUNK  # 4 col chunks per row-block

F32 = mybir.dt.float32
BF16 = mybir.dt.bfloat16
X = mybir.AxisListType.X
OP = mybir.AluOpType
AF = mybir.ActivationFunctionType


def _build() -> bass.Bass:
    nc = bacc.Bacc(
        "TRN2", target_bir_lowering=False, debug=False, num_devices=N_CORES
    )
    x_d = nc.declare_dram_parameter("x", [R, COLS], F32, isOutput=False)
    o_d = nc.declare_dram_parameter("out", [R, COLS], F32, isOutput=True)

    with ExitStack() as ctx:
        tc = ctx.enter_context(tile.TileContext(nc))
        xpool = ctx.enter_context(tc.tile_pool(name="xc", bufs=NCH))
        mpool = ctx.enter_context(tc.tile_pool(name="mc", bufs=NCH + 2))
        opool = ctx.enter_context(tc.tile_pool(name="oc", bufs=3))
        spool = ctx.enter_context(tc.tile_pool(name="sc", bufs=2))
        stats = ctx.enter_context(tc.tile_pool(name="stats", bufs=RB))

        for rb in range(RB):
            rows = slice(rb * P, (rb + 1) * P)
            xcs = []
            for c in range(NCH):
                cs = slice(c * CHUNK, (c + 1) * CHUNK)
                xc = xpool.tile([P, CHUNK], F32, tag="xc")
                nc.sync.dma_start(out=xc[:], in_=x_d[rows, cs])
                xcs.append(xc)

            abss = stats.tile([P, NCH], F32, tag="abss")
            npos = stats.tile([P, NCH], F32, tag="npos")
            sgns = stats.tile([P, NCH], F32, tag="sgns")

            mcs = []
            for c in range(NCH):
                sc = spool.tile([P, CHUNK], BF16, tag="sc")
                nc.scalar.activation(
                    out=sc[:], in_=xcs[c][:], func=AF.Abs,
                    accum_out=abss[:, c : c + 1],
                )
                sc2 = spool.tile([P, CHUNK], BF16, tag="sc")
                nc.scalar.activation(
                    out=sc2[:], in_=xcs[c][:], func=AF.Sign,
                    accum_out=sgns[:, c : c + 1],
                )
                mc = mpool.tile([P, CHUNK], BF16, tag="mc")
                nc.vector.tensor_scalar(
                    out=mc[:], in0=xcs[c][:], scalar1=0.0, scalar2=None,
                    op0=OP.is_gt, op1=OP.add,
                    accum_out=npos[:, c : c + 1],
                )
                mcs.append(mc)

            # Combine chunk partials -> per-row stats [P,1].
            absT = stats.tile([P, 1], F32, tag="absT")
            nc.vector.tensor_reduce(out=absT[:], in_=abss[:], axis=X, op=OP.add)
            nposT = stats.tile([P, 1], F32, tag="nposT")
            nc.vector.tensor_reduce(out=nposT[:], in_=npos[:], axis=X, op=OP.add)
            sgnT = stats.tile([P, 1], F32, tag="sgnT")
            nc.vector.tensor_reduce(out=sgnT[:], in_=sgns[:], axis=X, op=OP.add)

            # count = 2*npos - sgnsum  (= npos + nneg, exact integers in f32)
            cnt = stats.tile([P, 1], F32, tag="cnt")
            nc.vector.tensor_scalar(
                out=cnt[:], in0=nposT[:], scalar1=2.0, scalar2=sgnT[:],
                op0=OP.mult, op1=OP.subtract,
            )
            rcnt = stats.tile([P, 1], F32, tag="rcnt")
            nc.vector.reciprocal(rcnt[:], cnt[:])
            # alpha2 = 2 * abssum / count ; na = -alpha
            a2 = stats.tile([P, 1], F32, tag="a2")
            nc.vector.tensor_scalar(
                out=a2[:], in0=absT[:], scalar1=rcnt[:], scalar2=2.0,
                op0=OP.mult, op1=OP.mult,
            )
            na = stats.tile([P, 1], F32, tag="na")
            nc.vector.tensor_scalar(
                out=na[:], in0=a2[:], scalar1=-0.5, scalar2=None, op0=OP.mult,
            )

            for c in range(NCH):
                cs = slice(c * CHUNK, (c + 1) * CHUNK)
                # oc = mc*2alpha - alpha -> {+alpha, -alpha}
                oc = opool.tile([P, CHUNK], F32, tag="oc")
                nc.vector.tensor_scalar(
                    out=oc[:], in0=mcs[c][:], scalar1=a2[:], scalar2=na[:],
                    op0=OP.mult, op1=OP.add,
                )
                nc.sync.dma_start(out=o_d[rows, cs], in_=oc[:])

    nc.finalize()  # Bacc: runs compile() incl. sync-wait legalization
    return nc


_NC_CACHE = None


def _run(x: np.ndarray, trace: bool = False, trace_cores=None):
    global _NC_CACHE
    if _NC_CACHE is None:
        _NC_CACHE = _build()
    nc = _NC_CACHE
    x = np.ascontiguousarray(np.asarray(x, dtype=np.float32))
    assert x.shape == (ROWS, COLS), x.shape
    in_maps = [{"x": x[i * R : (i + 1) * R]} for i in range(N_CORES)]
    res = run_bass_kernel_spmd(
        nc, in_maps, list(range(N_CORES)), trace=trace, trace_cores=trace_cores
    )
    out = np.concatenate([res.results[i]["out"] for i in range(N_CORES)], axis=0)
    return out, res


def kernel(x: np.ndarray) -> np.ndarray:
    out, _ = _run(x)
    return out
